# revision 5
# baseline (speedup 1.0000x reference)
"""DiT attention block as a Bass/Tile kernel for 8 Trainium2 NeuronCores.

v4: K/V-projection dedup via pair-shared HBM exchange.

Sharding: core c -> batch b = c//2, sequence half = c%2; each core computes
output rows [half*1024, half*1024+1024) of batch b. The host rolls each
core's sequence so its own rows are rows [0, 1024).

Unlike v3 (which duplicated the K/V projection for the peer's half), each
core projects+norms+ropes+packs K and V only for its OWN 1024 rows, then
exchanges the packed kT / v16 halves with its pair partner through a
pair-shared HBM tensor (addr_space="Shared" maps the same physical HBM for
cores (2k, 2k+1)):

  scatter own rows -> [8-core barrier cc] -> gather peer rows

The indirect DMAs (per-partition row scatter/gather with host-supplied
per-core index tiles) keep the program rank-symmetric (no control flow).
Ordering is pure Tile dependency tracking: the scatter/gather declare
whole-tensor APs on the shared tensor, and tiny shuttle DMAs thread the
barrier collectives into that dependency chain. Two trailing barriers
protect the shared slots against overwrite across in-NEFF repeats.

Layout/dtype strategy (fp16 everywhere that streams the PE) is as v3:
fp16 QKV/out projections with persistent stationary x, rope in natural
layout, [128,128] PE pair-transposes into packed kT/qT, AV-stationary v16
with interleaved ones column accumulating the softmax denominator.
Q-phase rope runs entirely on DVE (vector) so the gpsimd queue is free for
the exchange chain.
"""

import sys

if "/opt/trn_rl_repo" not in sys.path:
    sys.path.insert(0, "/opt/trn_rl_repo")

from contextlib import ExitStack

import numpy as np

import concourse.bass as bass
import concourse.tile as tile
from concourse import mybir, bass_utils
from concourse.masks import make_identity
from concourse.vector_clock import ScopedClock, VectorClock

B, L, D, H = 4, 2048, 1024, 16
HD = D // H          # 64
HHD = HD // 2        # 32
EPS = 1e-6
THETA = 10000.0
N_CORES = 8
LQ = L // 2
P = 128
NCK = L // P         # 16
NCQ = LQ // P        # 8
NDC = D // P         # 8
KROW = NDC * LQ      # kT half: 8192 elems/partition
VROW = NCQ * H * (HD + 1)  # v16 half: 8320 elems/partition
F32 = mybir.dt.float32
FR = mybir.dt.float32r
F16 = mybir.dt.float16
I32 = mybir.dt.int32
AF = mybir.ActivationFunctionType
ALL8 = [list(range(N_CORES))]


def _patch_tile_drain():
    """This container's walrus rejects >1 sem wait per instruction.
    Tile's kernel-tail drain waits on every active proc at once; split those
    waits across single-wait NOPs on SP so the drain itself needs none."""
    if getattr(tile.TileContext, "_drain_split_patched", False):
        return

    def _patched(self, tick_clock, wait_clock):
        vc = tick_clock.global_clock
        n = len(vc)
        cur = VectorClock([0] * n)
        for proc in range(n):
            t = vc[proc]
            if t > 0:
                nop = self.nc.sync.nop(hint=f"drainsplit_{proc}", nofuse=True)
                req = VectorClock([0] * n)
                req.require_at_least(proc, t)
                wait_clock.add_sem_waits(
                    nop.ins, ScopedClock({None: req}), ScopedClock({None: cur.copy()})
                )
                cur.require_at_least(proc, t)
        drain_inst = self.nc.sync.drain()
        wait_clock.add_sem_waits(
            drain_inst.ins, ScopedClock({None: vc}), ScopedClock({None: cur})
        )
        self.nc.all_engine_barrier()
        popped = self.nc._tile_sem_poison_stack.pop()
        assert popped is self._sem_poison
        self.nc.clear_and_free_semaphores(list(self.sems.allocated().values()))
        self.nc.all_engine_barrier()

    tile.TileContext._drain_and_barrier = _patched
    tile.TileContext._drain_split_patched = True


def _split_waits(nc, maxw=1):
    """Hoist excess sem waits onto NOPs (walrus allows 1 wait/instruction)."""
    nid = 0
    for fn in nc.m.functions:
        for bb in fn.blocks:
            insts = list(bb.instructions)
            new = []
            changed = False
            for inst in insts:
                si = inst.sync_info
                if si is not None and si.on_wait is not None and len(si.on_wait) > maxw:
                    waits = list(si.on_wait)
                    extra, keep = waits[:-maxw], waits[-maxw:]
                    for i in range(0, len(extra), maxw):
                        nid += 1
                        new.append(mybir.InstNoOp(
                            name=f"I-wsplit-{nid}", engine=inst.engine,
                            sync_info=mybir.SyncInfo(
                                on_wait=extra[i : i + maxw], on_update=[]),
                        ))
                    inst.sync_info = mybir.SyncInfo(
                        on_wait=keep, on_update=list(si.on_update))
                    changed = True
                new.append(inst)
            if changed:
                bb.instructions = new


def _bcast_free(ap, repeat, at):
    new = ap.copy()
    new.ap = new.ap[: 1 + at] + [[0, repeat]] + new.ap[1 + at :]
    return new


def _build_program(use_bq, use_bk, use_bv, use_bout, use_qnw, use_knw,
                   repeat=1):
    nc = bass.Bass("TRN2", target_bir_lowering=False, debug=False,
                   num_devices=N_CORES)

    xt16 = nc.dram_tensor("xt16", [P, NDC, LQ], F16, kind="ExternalInput").ap()
    wq16 = nc.dram_tensor("wq16", [P, NDC, D], F16, kind="ExternalInput").ap()
    wk16 = nc.dram_tensor("wk16", [P, NDC, D], F16, kind="ExternalInput").ap()
    wv16 = nc.dram_tensor("wv16", [P, NDC, D], F16, kind="ExternalInput").ap()
    wob = nc.dram_tensor("wob", [P, NDC, D], F16, kind="ExternalInput").ap()
    cosq = nc.dram_tensor("cosq", [P, NCQ, HHD], F16, kind="ExternalInput").ap()
    sinq = nc.dram_tensor("sinq", [P, NCQ, HHD], F16, kind="ExternalInput").ap()
    idx_own = nc.dram_tensor("idx_own", [P, 1], I32, kind="ExternalInput").ap()
    idx_peer = nc.dram_tensor("idx_peer", [P, 1], I32,
                              kind="ExternalInput").ap()
    bq = bk = bv = bo = qnw = knw = None
    if use_bq:
        bq = nc.dram_tensor("bq", [1, D], F32, kind="ExternalInput").ap()
    if use_bk:
        bk = nc.dram_tensor("bk", [1, D], F32, kind="ExternalInput").ap()
    if use_bv:
        bv = nc.dram_tensor("bv", [1, D], F32, kind="ExternalInput").ap()
    if use_bout:
        bo = nc.dram_tensor("bout", [1, D], F32, kind="ExternalInput").ap()
    if use_qnw:
        qnw = nc.dram_tensor("qnw", [1, HD], F32, kind="ExternalInput").ap()
    if use_knw:
        knw = nc.dram_tensor("knw", [1, HD], F32, kind="ExternalInput").ap()
    out = nc.dram_tensor("out", [LQ, D], F32, kind="ExternalOutput").ap()

    # pair-shared HBM exchange tensors: rows 0..127 even-core half,
    # 128..255 odd-core half, row 256 barrier shuttle scratch
    kxch = nc.dram_tensor("kxch", [2 * P + 1, KROW], F16,
                          addr_space="Shared").ap()
    vxch = nc.dram_tensor("vxch", [2 * P + 1, VROW], F16,
                          addr_space="Shared").ap()
    bar_in = nc.dram_tensor("bar_in", [1, 16], F16).ap()
    bar_out = nc.dram_tensor("bar_out", [N_CORES, 16], F16).ap()

    with tile.TileContext(nc) as tc, ExitStack() as ctx:
        pers = ctx.enter_context(tc.tile_pool(name="pers", bufs=1))
        dpool = ctx.enter_context(tc.tile_pool(name="dram", bufs=1, space="DRAM"))
        invstage = dpool.tile([H, LQ], F32, tag="invstage")

        identh = pers.tile([P, P], F16, tag="identh")
        identf = pers.tile([P, P], F32, tag="identf")
        make_identity(nc, identf)
        nc.vector.tensor_copy(identh, identf)

        xt16_sb = pers.tile([P, NDC, LQ], F16, tag="xt16")
        wk_first = pers.tile([P, NDC, D], F16, tag="wkf")
        cosq_sb = pers.tile([P, NCQ, HHD], F16, tag="cosq")
        sinq_sb = pers.tile([P, NCQ, HHD], F16, tag="sinq")
        io_sb = pers.tile([P, 1], I32, tag="io_sb")
        ip_sb = pers.tile([P, 1], I32, tag="ip_sb")
        nc.sync.dma_start(out=xt16_sb[:, :, 0:512], in_=xt16[:, :, 0:512])
        nc.sync.dma_start(out=wk_first, in_=wk16)
        nc.sync.dma_start(out=cosq_sb, in_=cosq)
        nc.sync.dma_start(out=sinq_sb, in_=sinq)
        nc.sync.dma_start(out=io_sb, in_=idx_own)
        nc.sync.dma_start(out=ip_sb, in_=idx_peer)
        nc.sync.dma_start(out=xt16_sb[:, :, 512:LQ], in_=xt16[:, :, 512:LQ])

        # half-major so both exchange slices are contiguous per partition
        kT = pers.tile([P, 2, H // 2, LQ], F16, tag="kT")
        kTflat = kT.rearrange("p s i l -> p (s i l)")
        qT = pers.tile([P, H // 2, LQ], F16, tag="qT")
        # v16[:, ci, h, 0:64] = v chunk; [..., 64] = 1.0 so the AV matmul's
        # stationary [128, 65] slice also accumulates the softmax denominator
        v16 = pers.tile([P, NCK, H, HD + 1], F16, tag="v16")
        nc.vector.memset(v16[:, :, :, HD : HD + 1], 1.0)
        attnT = pers.tile([P, H // 2, LQ], F16, tag="attnT")

        eps_sb = pers.tile([P, 1], F32, tag="eps")
        nc.vector.memset(eps_sb, EPS)
        ones1 = None
        if use_bq or use_bk or use_bv:
            ones1f = pers.tile([1, P], F32, tag="ones1f")
            nc.vector.memset(ones1f, 1.0)
            ones1 = pers.tile([1, P], FR, tag="ones1")
            nc.vector.tensor_copy(ones1, ones1f)
        qnw_b = knw_b = bout_b = None
        if use_qnw:
            qnw_b = pers.tile([P, HD], F32, tag="qnw_b")
            nc.sync.dma_start(
                out=qnw_b,
                in_=bass.AP(tensor=qnw.tensor, offset=qnw.offset,
                            ap=[[0, P], [1, HD]]),
            )
        if use_knw:
            knw_b = pers.tile([P, HD], F32, tag="knw_b")
            nc.sync.dma_start(
                out=knw_b,
                in_=bass.AP(tensor=knw.tensor, offset=knw.offset,
                            ap=[[0, P], [1, HD]]),
            )
        if use_bout:
            bout_b = pers.tile([P, D], F32, tag="bout_b")
            nc.sync.dma_start(
                out=bout_b,
                in_=bass.AP(tensor=bo.tensor, offset=bo.offset,
                            ap=[[0, P], [1, D]]),
            )

        def load_bias(pool, b_dram, tag):
            b_sb = pool.tile([1, D], FR, tag=tag)
            nc.sync.dma_start(out=b_sb, in_=b_dram.bitcast(FR))
            return b_sb

        def proj_chunk(ps, l0, w_sb, b_sb):
            """ps[128, D] (PSUM f32) = x[l0:l0+128, :] @ W (fp16 operands).
            l0 in units of rows of this core's own half."""
            for n0 in range(0, D, 512):
                for j in range(NDC):
                    nc.tensor.matmul(
                        ps[:, n0 : n0 + 512],
                        xt16_sb[:, j, l0 : l0 + P],
                        w_sb[:, j, n0 : n0 + 512],
                        start=(j == 0),
                        stop=(j == NDC - 1 and b_sb is None),
                    )
                if b_sb is not None:
                    nc.tensor.matmul(
                        ps[:, n0 : n0 + 512],
                        ones1,
                        b_sb[:, n0 : n0 + 512],
                        start=False,
                        stop=True,
                    )
            return ps

        def norm_rope(ps, cos_ap, sin_ap, nw_b, stg, mul1, mul2):
            """RMSNorm + rope from PSUM [128, D]; returns fp16 [128, H, HD].
            mul1/mul2: engines for the first two rope products (gpsimd during
            the K phase, vector during Q so gpsimd stays free for the
            exchange chain)."""
            sq = stg.tile([P, D], F32, tag="sq")
            nc.scalar.activation(sq, ps, AF.Square)
            ss = stg.tile([P, H], F32, tag="ss")
            nc.vector.tensor_reduce(
                ss, sq.rearrange("p (h d) -> p h d", h=H),
                axis=mybir.AxisListType.X, op=mybir.AluOpType.add,
            )
            inv = stg.tile([P, H], F32, tag="inv")
            nc.scalar.activation(inv, ss, AF.Sqrt, scale=1.0 / HD, bias=eps_sb)
            nc.vector.reciprocal(inv, inv)
            ps3 = ps.rearrange("p (h d) -> p h d", h=H)
            kn = stg.tile([P, H, HD], F32, tag="kn")
            nc.vector.tensor_mul(kn, ps3, _bcast_free(inv, HD, 1))
            if nw_b is not None:
                nc.vector.tensor_mul(kn, kn, _bcast_free(nw_b, H, 0))
            t1 = kn[:, :, 0:HHD]
            t2 = kn[:, :, HHD:HD]
            cosc = _bcast_free(cos_ap, H, 0)
            sinc = _bcast_free(sin_ap, H, 0)
            ra = stg.tile([P, H, HHD], F16, tag="ra")
            rb = stg.tile([P, H, HHD], F16, tag="rb")
            rc = stg.tile([P, H, HHD], F16, tag="rc")
            rd = stg.tile([P, H, HHD], F16, tag="rd")
            rot = stg.tile([P, H, HD], F16, tag="rot")
            mul1.tensor_mul(ra, t1, cosc)
            mul2.tensor_mul(rb, t2, sinc)
            nc.vector.tensor_sub(rot[:, :, 0:HHD], ra, rb)
            mul1.tensor_mul(rc, t1, sinc)
            nc.vector.tensor_mul(rd, t2, cosc)
            nc.vector.tensor_add(rot[:, :, HHD:HD], rc, rd)
            return rot

        def transpose_pairs(rot, dstT, ci, tppool):
            """[128, 128] fp16 PE transposes: head pair p -> packed layout."""
            tp = tppool.tile([P, H // 2, P], F16, tag="tp")
            for p in range(H // 2):
                nc.tensor.transpose(
                    tp[:, p, :],
                    rot.rearrange("p h d -> p (h d)")[:, p * P : (p + 1) * P],
                    identh,
                )
            nc.scalar.copy(dstT[:, :, ci * P : (ci + 1) * P], tp)

        def exchange(xch, sb_own, sb_peer, final_bars):
            """Scatter own rows into shared HBM, barrier, gather peer rows.
            All ordering is Tile dependency tracking: the indirect DMAs
            declare whole-tensor APs on xch, and the shuttle DMAs thread the
            barrier collectives into that chain."""
            nc.gpsimd.indirect_dma_start(
                out=xch, out_offset=bass.IndirectOffsetOnAxis(
                    ap=io_sb[:, 0:1], axis=0),
                in_=sb_own, in_offset=None,
            )
            nc.gpsimd.dma_start(out=bar_in, in_=xch[2 * P : 2 * P + 1, 0:16])
            nc.gpsimd.collective_compute(
                "AllGather", mybir.AluOpType.bypass, replica_groups=ALL8,
                ins=[bar_in], outs=[bar_out],
            )
            nc.gpsimd.dma_start(out=xch[2 * P : 2 * P + 1, 16:32],
                                in_=bar_out[0:1, 0:16])
            nc.gpsimd.indirect_dma_start(
                out=sb_peer, out_offset=None,
                in_=xch, in_offset=bass.IndirectOffsetOnAxis(
                    ap=ip_sb[:, 0:1], axis=0),
            )
            if final_bars:
                # reads done on both cores -> next repeat may overwrite slots
                for xch2, sbp in final_bars:
                    nc.gpsimd.dma_start(out=bar_in, in_=sbp)
                    nc.gpsimd.collective_compute(
                        "AllGather", mybir.AluOpType.bypass,
                        replica_groups=ALL8, ins=[bar_in], outs=[bar_out],
                    )
                    nc.gpsimd.dma_start(
                        out=xch2[2 * P : 2 * P + 1, 32:48],
                        in_=bar_out[0:1, 0:16])

        for _rep in range(repeat):
            # ---- Phase A: K own half -> exchange; V own half -> exchange;
            #      Q own half. ----
            with ExitStack() as ph:
                wpool = ph.enter_context(tc.tile_pool(name="wA", bufs=1))
                pspool = ph.enter_context(
                    tc.tile_pool(name="psA", bufs=3, space="PSUM"))
                tppool = ph.enter_context(
                    tc.tile_pool(name="tpA", bufs=2, space="PSUM"))
                stg = ph.enter_context(tc.tile_pool(name="stgA", bufs=2))
                bq_sb = load_bias(pers, bq, "bq_sb") if use_bq else None
                bk_sb = load_bias(pers, bk, "bk_sb") if use_bk else None
                bv_sb = load_bias(pers, bv, "bv_sb") if use_bv else None

                for ci in range(NCQ):
                    psk = pspool.tile([P, D], F32, tag="ps")
                    proj_chunk(psk, ci * P, wk_first, bk_sb)
                    rotk = norm_rope(psk, cosq_sb[:, ci, :], sinq_sb[:, ci, :],
                                     knw_b, stg, nc.gpsimd, nc.gpsimd)
                    transpose_pairs(rotk, kT[:, 0], ci, tppool)

                exchange(kxch, kTflat[:, 0:KROW], kTflat[:, KROW : 2 * KROW],
                         None)

                wv_sb = wpool.tile([P, NDC, D], F16, tag="w")
                nc.sync.dma_start(out=wv_sb, in_=wv16)
                for ci in range(NCQ):
                    psv = pspool.tile([P, D], F32, tag="ps")
                    proj_chunk(psv, ci * P, wv_sb, bv_sb)
                    nc.scalar.copy(
                        v16[:, ci, :, 0:HD],
                        psv.rearrange("p (h d) -> p h d", h=H),
                    )

                vflat = v16.rearrange("p c h d -> p (c h d)")
                exchange(
                    vxch, vflat[:, 0:VROW], vflat[:, VROW : 2 * VROW],
                    [(kxch, kTflat[0:1, 2 * KROW - 16 : 2 * KROW]),
                     (vxch, vflat[0:1, 2 * VROW - 16 : 2 * VROW])],
                )

                wq_sb = wpool.tile([P, NDC, D], F16, tag="w")
                nc.sync.dma_start(out=wq_sb, in_=wq16)
                for ci in range(NCQ):
                    psq = pspool.tile([P, D], F32, tag="ps")
                    proj_chunk(psq, ci * P, wq_sb, bq_sb)
                    rotq = norm_rope(psq, cosq_sb[:, ci, :], sinq_sb[:, ci, :],
                                     qnw_b, stg, nc.vector, nc.vector)
                    transpose_pairs(rotq, qT, ci, tppool)

            # ---- Phase B: attention ----
            with ExitStack() as ph:
                spool = ph.enter_context(
                    tc.tile_pool(name="sB", bufs=2, space="PSUM"))
                upool = ph.enter_context(
                    tc.tile_pool(name="uB", bufs=2, space="PSUM"))
                ppool = ph.enter_context(tc.tile_pool(name="ptB", bufs=4))
                bcpool = ph.enter_context(tc.tile_pool(name="bcB", bufs=2))

                def s_chunk(h, c, sT):
                    pi, po = h // 2, (h % 2) * HD
                    for n0 in range(0, LQ, 512):
                        nc.tensor.matmul(
                            sT[:, n0 : n0 + 512],
                            kT[po : po + HD, c // NCQ, pi,
                               (c % NCQ) * P : (c % NCQ + 1) * P],
                            qT[po : po + HD, pi, n0 : n0 + 512],
                            start=True,
                            stop=True,
                        )

                def av_chunk(h, c, pt, U):
                    for n0 in range(0, LQ, 512):
                        nc.tensor.matmul(
                            U[:, n0 : n0 + 512],
                            v16[:, c, h, :],
                            pt[:, n0 : n0 + 512],
                            start=(c == 0),
                            stop=(c == NCK - 1),
                        )

                for h in range(H):
                    pi = h // 2
                    U = upool.tile([HD + 1, LQ], F32, tag="U")
                    pt_prev = None
                    for c in range(NCK):
                        sT = spool.tile([P, LQ], F32, tag="sT")
                        s_chunk(h, c, sT)
                        pt = ppool.tile([P, LQ], F16, tag="pt")
                        nc.scalar.activation(pt, sT, AF.Exp, scale=0.125)
                        if pt_prev is not None:
                            av_chunk(h, c - 1, pt_prev, U)
                        pt_prev = pt
                    av_chunk(h, NCK - 1, pt_prev, U)
                    po = (h % 2) * HD
                    nc.scalar.copy(attnT[po : po + HD, pi, :], U[0:HD, :])
                    inv = bcpool.tile([1, LQ], F32, tag="inv")
                    nc.vector.reciprocal(inv, U[HD : HD + 1, :])
                    nc.sync.dma_start(out=invstage[h, :], in_=inv)
                    if h % 2 == 1:
                        bc = bcpool.tile([P, LQ], F32, tag="bc")
                        for hh in range(2):
                            iv = invstage[2 * pi + hh, :]
                            nc.sync.dma_start(
                                out=bc[hh * HD : (hh + 1) * HD, :],
                                in_=bass.AP(tensor=iv.tensor, offset=iv.offset,
                                            ap=[[0, HD], [1, LQ]]),
                            )
                        nc.vector.tensor_mul(attnT[:, pi, :], attnT[:, pi, :], bc)

            # ---- Phase C: out projection ----
            with ExitStack() as ph:
                opool = ph.enter_context(
                    tc.tile_pool(name="oC", bufs=2, space="PSUM"))
                obpool = ph.enter_context(tc.tile_pool(name="obC", bufs=2))
                # reuse the K-weights buffer for Wout (disjoint lifetimes)
                wout_sb = wk_first
                nc.sync.dma_start(out=wout_sb, in_=wob)
                for cj in range(NCQ):
                    pso = opool.tile([P, D], F32, tag="pso")
                    for n0 in range(0, D, 512):
                        for j in range(NDC):
                            nc.tensor.matmul(
                                pso[:, n0 : n0 + 512],
                                attnT[:, j, cj * P : (cj + 1) * P],
                                wout_sb[:, j, n0 : n0 + 512],
                                start=(j == 0),
                                stop=(j == NDC - 1),
                            )
                    ob = obpool.tile([P, D], F32, tag="ob")
                    if use_bout:
                        nc.vector.tensor_add(ob, pso, bout_b)
                    else:
                        nc.scalar.copy(ob[:, 0 : D // 2], pso[:, 0 : D // 2])
                        nc.vector.tensor_copy(ob[:, D // 2 : D], pso[:, D // 2 : D])
                    nc.sync.dma_start(out=out[cj * P : (cj + 1) * P, :], in_=ob)

    return nc


_PROGRAM_CACHE = {}


def _get_program(flags, repeat=1):
    key = (flags, repeat)
    if key not in _PROGRAM_CACHE:
        _patch_tile_drain()
        _PROGRAM_CACHE[key] = _build_program(*flags, repeat=repeat)
    return _PROGRAM_CACHE[key]


def _rope_tables():
    pos = np.arange(L, dtype=np.float32)
    inv_freq = (1.0 / (THETA ** (np.arange(0, HD, 2, dtype=np.float32) / HD))
                ).astype(np.float32)
    ang = pos[:, None] * inv_freq[None, :]
    return np.cos(ang).astype(np.float32), np.sin(ang).astype(np.float32)


def _chunked_pf(t, nch):
    """[nch*128, F] -> [128, nch, F] (partition-major chunk layout)."""
    return np.ascontiguousarray(
        t.reshape(nch, P, -1).transpose(1, 0, 2))


def _w16_layout(w):
    """[D, N] f32 -> [128, NDC, N] fp16."""
    return np.ascontiguousarray(
        w.reshape(NDC, P, -1).transpose(1, 0, 2)).astype(np.float16)


def _make_in_maps(x, Wqkv, bqkv, qn_w, kn_w, Wout, bout, flags):
    use_bq, use_bk, use_bv, use_bout, use_qnw, use_knw = flags
    cos, sin = _rope_tables()
    cosh = cos.astype(np.float16)
    sinh = sin.astype(np.float16)
    wq16 = _w16_layout(Wqkv[:, 0:D])
    wk16 = _w16_layout(Wqkv[:, D : 2 * D])
    wv16 = _w16_layout(Wqkv[:, 2 * D : 3 * D])
    wob = _w16_layout(Wout)
    base_idx = np.arange(P, dtype=np.int32).reshape(P, 1)
    in_maps = []
    for c in range(N_CORES):
        b, half = c // 2, c % 2
        # roll the sequence so this core's own rows are rows [0, LQ);
        # rope tables roll identically (softmax over k is order-invariant)
        xr = np.roll(x[b], -half * LQ, axis=0)[0:LQ]
        cosr = np.roll(cosh, -half * LQ, axis=0)[0:LQ]
        sinr = np.roll(sinh, -half * LQ, axis=0)[0:LQ]
        xt16 = np.ascontiguousarray(
            xr.T.reshape(NDC, P, LQ).transpose(1, 0, 2)).astype(np.float16)
        m = {
            "xt16": xt16,
            "wq16": wq16,
            "wk16": wk16,
            "wv16": wv16,
            "wob": wob,
            "cosq": _chunked_pf(cosr, NCQ),
            "sinq": _chunked_pf(sinr, NCQ),
            "idx_own": base_idx + half * P,
            "idx_peer": base_idx + (1 - half) * P,
        }
        if use_bq:
            m["bq"] = np.ascontiguousarray(bqkv[0:D]).reshape(1, D)
        if use_bk:
            m["bk"] = np.ascontiguousarray(bqkv[D : 2 * D]).reshape(1, D)
        if use_bv:
            m["bv"] = np.ascontiguousarray(bqkv[2 * D : 3 * D]).reshape(1, D)
        if use_bout:
            m["bout"] = np.ascontiguousarray(bout).reshape(1, D)
        if use_qnw:
            m["qnw"] = np.ascontiguousarray(qn_w).reshape(1, HD)
        if use_knw:
            m["knw"] = np.ascontiguousarray(kn_w).reshape(1, HD)
        in_maps.append(m)
    return in_maps


def _flags_for(bqkv, qn_w, kn_w, bout):
    return (
        bool(np.any(bqkv[0:D])),
        bool(np.any(bqkv[D : 2 * D])),
        bool(np.any(bqkv[2 * D : 3 * D])),
        bool(np.any(bout)),
        bool(np.any(qn_w != 1.0)),
        bool(np.any(kn_w != 1.0)),
    )


def _assemble(results):
    out = np.empty((B, L, D), dtype=np.float32)
    for c in range(N_CORES):
        b, half = c // 2, c % 2
        out[b, half * LQ : (half + 1) * LQ, :] = results[c]["out"]
    return out


def kernel(x, Wqkv, bqkv, qn_w, kn_w, Wout, bout, _trace=False):
    x = np.asarray(x, dtype=np.float32)
    Wqkv = np.asarray(Wqkv, dtype=np.float32)
    bqkv = np.asarray(bqkv, dtype=np.float32)
    qn_w = np.asarray(qn_w, dtype=np.float32)
    kn_w = np.asarray(kn_w, dtype=np.float32)
    Wout = np.asarray(Wout, dtype=np.float32)
    bout = np.asarray(bout, dtype=np.float32)

    flags = _flags_for(bqkv, qn_w, kn_w, bout)
    nc = _get_program(flags)
    if not getattr(nc, "_waits_split", False):
        _split_waits(nc)
        nc._waits_split = True
    in_maps = _make_in_maps(x, Wqkv, bqkv, qn_w, kn_w, Wout, bout, flags)
    res = bass_utils.run_bass_kernel_spmd(
        nc, in_maps, core_ids=list(range(N_CORES))
    )
    out = _assemble(res.results)
    if _trace:
        return out, res
    return out


# revision 7
# speedup vs baseline: 2.0869x; 2.0869x over previous
"""DiT attention block as a Bass/Tile kernel for 8 Trainium2 NeuronCores.

v4: K/V-projection dedup via pair-shared HBM exchange.

Sharding: core c -> batch b = c//2, sequence half = c%2; each core computes
output rows [half*1024, half*1024+1024) of batch b. The host rolls each
core's sequence so its own rows are rows [0, 1024).

Unlike v3 (which duplicated the K/V projection for the peer's half), each
core projects+norms+ropes+packs K and V only for its OWN 1024 rows, then
exchanges the packed kT / v16 halves with its pair partner through a
pair-shared HBM tensor (addr_space="Shared" maps the same physical HBM for
cores (2k, 2k+1)):

  scatter own rows -> [8-core barrier cc] -> gather peer rows

The indirect DMAs (per-partition row scatter/gather with host-supplied
per-core index tiles) keep the program rank-symmetric (no control flow).
Ordering is pure Tile dependency tracking: the scatter/gather declare
whole-tensor APs on the shared tensor, and tiny shuttle DMAs thread the
barrier collectives into that dependency chain. Two trailing barriers
protect the shared slots against overwrite across in-NEFF repeats.

Layout/dtype strategy (fp16 everywhere that streams the PE) is as v3:
fp16 QKV/out projections with persistent stationary x, rope in natural
layout, [128,128] PE pair-transposes into packed kT/qT, AV-stationary v16
with interleaved ones column accumulating the softmax denominator.
Q-phase rope runs entirely on DVE (vector) so the gpsimd queue is free for
the exchange chain.
"""

import sys

if "/opt/trn_rl_repo" not in sys.path:
    sys.path.insert(0, "/opt/trn_rl_repo")

from contextlib import ExitStack

import numpy as np

import concourse.bass as bass
import concourse.tile as tile
from concourse import mybir, bass_utils
from concourse.masks import make_identity
from concourse.vector_clock import ScopedClock, VectorClock

B, L, D, H = 4, 2048, 1024, 16
HD = D // H          # 64
HHD = HD // 2        # 32
EPS = 1e-6
THETA = 10000.0
N_CORES = 8
LQ = L // 2
P = 128
NCK = L // P         # 16
NCQ = LQ // P        # 8
NDC = D // P         # 8
KROW = NDC * LQ      # kT half: 8192 elems/partition
VROW = NCQ * H * (HD + 1)  # v16 half: 8320 elems/partition
F32 = mybir.dt.float32
FR = mybir.dt.float32r
F16 = mybir.dt.float16
I32 = mybir.dt.int32
AF = mybir.ActivationFunctionType
ALL8 = [list(range(N_CORES))]


def _patch_tile_drain():
    """This container's walrus rejects >1 sem wait per instruction.
    Tile's kernel-tail drain waits on every active proc at once; split those
    waits across single-wait NOPs on SP so the drain itself needs none."""
    if getattr(tile.TileContext, "_drain_split_patched", False):
        return

    def _patched(self, tick_clock, wait_clock):
        vc = tick_clock.global_clock
        n = len(vc)
        cur = VectorClock([0] * n)
        for proc in range(n):
            t = vc[proc]
            if t > 0:
                nop = self.nc.sync.nop(hint=f"drainsplit_{proc}", nofuse=True)
                req = VectorClock([0] * n)
                req.require_at_least(proc, t)
                wait_clock.add_sem_waits(
                    nop.ins, ScopedClock({None: req}), ScopedClock({None: cur.copy()})
                )
                cur.require_at_least(proc, t)
        drain_inst = self.nc.sync.drain()
        wait_clock.add_sem_waits(
            drain_inst.ins, ScopedClock({None: vc}), ScopedClock({None: cur})
        )
        self.nc.all_engine_barrier()
        popped = self.nc._tile_sem_poison_stack.pop()
        assert popped is self._sem_poison
        self.nc.clear_and_free_semaphores(list(self.sems.allocated().values()))
        self.nc.all_engine_barrier()

    tile.TileContext._drain_and_barrier = _patched
    tile.TileContext._drain_split_patched = True


def _split_waits(nc, maxw=1):
    """Hoist excess sem waits onto NOPs (walrus allows 1 wait/instruction)."""
    nid = 0
    for fn in nc.m.functions:
        for bb in fn.blocks:
            insts = list(bb.instructions)
            new = []
            changed = False
            for inst in insts:
                si = inst.sync_info
                if si is not None and si.on_wait is not None and len(si.on_wait) > maxw:
                    waits = list(si.on_wait)
                    extra, keep = waits[:-maxw], waits[-maxw:]
                    for i in range(0, len(extra), maxw):
                        nid += 1
                        new.append(mybir.InstNoOp(
                            name=f"I-wsplit-{nid}", engine=inst.engine,
                            sync_info=mybir.SyncInfo(
                                on_wait=extra[i : i + maxw], on_update=[]),
                        ))
                    inst.sync_info = mybir.SyncInfo(
                        on_wait=keep, on_update=list(si.on_update))
                    changed = True
                new.append(inst)
            if changed:
                bb.instructions = new


def _bcast_free(ap, repeat, at):
    new = ap.copy()
    new.ap = new.ap[: 1 + at] + [[0, repeat]] + new.ap[1 + at :]
    return new


def _build_program(use_bq, use_bk, use_bv, use_bout, use_qnw, use_knw,
                   repeat=1, n_bars=2):
    nc = bass.Bass("TRN2", target_bir_lowering=False, debug=False,
                   num_devices=N_CORES)

    xt16 = nc.dram_tensor("xt16", [P, NDC, LQ], F16, kind="ExternalInput").ap()
    wq16 = nc.dram_tensor("wq16", [P, NDC, D], F16, kind="ExternalInput").ap()
    wk16 = nc.dram_tensor("wk16", [P, NDC, D], F16, kind="ExternalInput").ap()
    wv16 = nc.dram_tensor("wv16", [P, NDC, D], F16, kind="ExternalInput").ap()
    wob = nc.dram_tensor("wob", [P, NDC, D], F16, kind="ExternalInput").ap()
    cosq = nc.dram_tensor("cosq", [P, NCQ, HHD], F16, kind="ExternalInput").ap()
    sinq = nc.dram_tensor("sinq", [P, NCQ, HHD], F16, kind="ExternalInput").ap()
    idx_own = nc.dram_tensor("idx_own", [P, 1], I32, kind="ExternalInput").ap()
    idx_peer = nc.dram_tensor("idx_peer", [P, 1], I32,
                              kind="ExternalInput").ap()
    bq = bk = bv = bo = qnw = knw = None
    if use_bq:
        bq = nc.dram_tensor("bq", [1, D], F32, kind="ExternalInput").ap()
    if use_bk:
        bk = nc.dram_tensor("bk", [1, D], F32, kind="ExternalInput").ap()
    if use_bv:
        bv = nc.dram_tensor("bv", [1, D], F32, kind="ExternalInput").ap()
    if use_bout:
        bo = nc.dram_tensor("bout", [1, D], F32, kind="ExternalInput").ap()
    if use_qnw:
        qnw = nc.dram_tensor("qnw", [1, HD], F32, kind="ExternalInput").ap()
    if use_knw:
        knw = nc.dram_tensor("knw", [1, HD], F32, kind="ExternalInput").ap()
    out = nc.dram_tensor("out", [LQ, D], F32, kind="ExternalOutput").ap()

    # pair-shared HBM exchange tensors: rows 0..127 even-core half,
    # 128..255 odd-core half, row 256 barrier shuttle scratch
    kxch = nc.dram_tensor("kxch", [2 * P + 1, KROW], F16,
                          addr_space="Shared").ap()
    vxch = nc.dram_tensor("vxch", [2 * P + 1, VROW], F16,
                          addr_space="Shared").ap()
    bar_in = nc.dram_tensor("bar_in", [1, 16], F16).ap()
    bar_out = nc.dram_tensor("bar_out", [N_CORES, 16], F16).ap()

    with tile.TileContext(nc) as tc, ExitStack() as ctx:
        pers = ctx.enter_context(tc.tile_pool(name="pers", bufs=1))
        dpool = ctx.enter_context(tc.tile_pool(name="dram", bufs=1, space="DRAM"))
        invstage = dpool.tile([H, LQ], F32, tag="invstage")

        identh = pers.tile([P, P], F16, tag="identh")
        identf = pers.tile([P, P], F32, tag="identf")
        make_identity(nc, identf)
        nc.vector.tensor_copy(identh, identf)

        xt16_sb = pers.tile([P, NDC, LQ], F16, tag="xt16")
        wk_first = pers.tile([P, NDC, D], F16, tag="wkf")
        cosq_sb = pers.tile([P, NCQ, HHD], F16, tag="cosq")
        sinq_sb = pers.tile([P, NCQ, HHD], F16, tag="sinq")
        io_sb = pers.tile([P, 1], I32, tag="io_sb")
        ip_sb = pers.tile([P, 1], I32, tag="ip_sb")
        nc.sync.dma_start(out=xt16_sb[:, :, 0:512], in_=xt16[:, :, 0:512])
        nc.sync.dma_start(out=wk_first, in_=wk16)
        nc.sync.dma_start(out=cosq_sb, in_=cosq)
        nc.sync.dma_start(out=sinq_sb, in_=sinq)
        nc.sync.dma_start(out=io_sb, in_=idx_own)
        nc.sync.dma_start(out=ip_sb, in_=idx_peer)
        nc.sync.dma_start(out=xt16_sb[:, :, 512:LQ], in_=xt16[:, :, 512:LQ])

        # half-major so both exchange slices are contiguous per partition
        kT = pers.tile([P, 2, H // 2, LQ], F16, tag="kT")
        kTflat = kT.rearrange("p s i l -> p (s i l)")
        qT = pers.tile([P, H // 2, LQ], F16, tag="qT")
        # v16[:, ci, h, 0:64] = v chunk; [..., 64] = 1.0 so the AV matmul's
        # stationary [128, 65] slice also accumulates the softmax denominator
        v16 = pers.tile([P, NCK, H, HD + 1], F16, tag="v16")
        nc.vector.memset(v16[:, :, :, HD : HD + 1], 1.0)
        attnT = pers.tile([P, H // 2, LQ], F16, tag="attnT")

        eps_sb = pers.tile([P, 1], F32, tag="eps")
        nc.vector.memset(eps_sb, EPS)
        ones1 = None
        if use_bq or use_bk or use_bv:
            ones1f = pers.tile([1, P], F32, tag="ones1f")
            nc.vector.memset(ones1f, 1.0)
            ones1 = pers.tile([1, P], FR, tag="ones1")
            nc.vector.tensor_copy(ones1, ones1f)
        qnw_b = knw_b = bout_b = None
        if use_qnw:
            qnw_b = pers.tile([P, HD], F32, tag="qnw_b")
            nc.sync.dma_start(
                out=qnw_b,
                in_=bass.AP(tensor=qnw.tensor, offset=qnw.offset,
                            ap=[[0, P], [1, HD]]),
            )
        if use_knw:
            knw_b = pers.tile([P, HD], F32, tag="knw_b")
            nc.sync.dma_start(
                out=knw_b,
                in_=bass.AP(tensor=knw.tensor, offset=knw.offset,
                            ap=[[0, P], [1, HD]]),
            )
        if use_bout:
            bout_b = pers.tile([P, D], F32, tag="bout_b")
            nc.sync.dma_start(
                out=bout_b,
                in_=bass.AP(tensor=bo.tensor, offset=bo.offset,
                            ap=[[0, P], [1, D]]),
            )

        def load_bias(pool, b_dram, tag):
            b_sb = pool.tile([1, D], FR, tag=tag)
            nc.sync.dma_start(out=b_sb, in_=b_dram.bitcast(FR))
            return b_sb

        def proj_chunk(ps, l0, w_sb, b_sb):
            """ps[128, D] (PSUM f32) = x[l0:l0+128, :] @ W (fp16 operands).
            l0 in units of rows of this core's own half."""
            for n0 in range(0, D, 512):
                for j in range(NDC):
                    nc.tensor.matmul(
                        ps[:, n0 : n0 + 512],
                        xt16_sb[:, j, l0 : l0 + P],
                        w_sb[:, j, n0 : n0 + 512],
                        start=(j == 0),
                        stop=(j == NDC - 1 and b_sb is None),
                    )
                if b_sb is not None:
                    nc.tensor.matmul(
                        ps[:, n0 : n0 + 512],
                        ones1,
                        b_sb[:, n0 : n0 + 512],
                        start=False,
                        stop=True,
                    )
            return ps

        def norm_rope(ps, cos_ap, sin_ap, nw_b, stg, mul1, mul2):
            """RMSNorm + rope from PSUM [128, D]; returns fp16 [128, H, HD].
            mul1/mul2: engines for the first two rope products (gpsimd during
            the K phase, vector during Q so gpsimd stays free for the
            exchange chain)."""
            sq = stg.tile([P, D], F32, tag="sq")
            nc.scalar.activation(sq, ps, AF.Square)
            ss = stg.tile([P, H], F32, tag="ss")
            nc.vector.tensor_reduce(
                ss, sq.rearrange("p (h d) -> p h d", h=H),
                axis=mybir.AxisListType.X, op=mybir.AluOpType.add,
            )
            inv = stg.tile([P, H], F32, tag="inv")
            nc.scalar.activation(inv, ss, AF.Sqrt, scale=1.0 / HD, bias=eps_sb)
            nc.vector.reciprocal(inv, inv)
            ps3 = ps.rearrange("p (h d) -> p h d", h=H)
            kn = stg.tile([P, H, HD], F32, tag="kn")
            nc.vector.tensor_mul(kn, ps3, _bcast_free(inv, HD, 1))
            if nw_b is not None:
                nc.vector.tensor_mul(kn, kn, _bcast_free(nw_b, H, 0))
            t1 = kn[:, :, 0:HHD]
            t2 = kn[:, :, HHD:HD]
            cosc = _bcast_free(cos_ap, H, 0)
            sinc = _bcast_free(sin_ap, H, 0)
            ra = stg.tile([P, H, HHD], F16, tag="ra")
            rb = stg.tile([P, H, HHD], F16, tag="rb")
            rc = stg.tile([P, H, HHD], F16, tag="rc")
            rd = stg.tile([P, H, HHD], F16, tag="rd")
            rot = stg.tile([P, H, HD], F16, tag="rot")
            mul1.tensor_mul(ra, t1, cosc)
            mul2.tensor_mul(rb, t2, sinc)
            nc.vector.tensor_sub(rot[:, :, 0:HHD], ra, rb)
            mul1.tensor_mul(rc, t1, sinc)
            nc.vector.tensor_mul(rd, t2, cosc)
            nc.vector.tensor_add(rot[:, :, HHD:HD], rc, rd)
            return rot

        def transpose_pairs(rot, dstT, ci, tppool):
            """[128, 128] fp16 PE transposes: head pair p -> packed layout."""
            tp = tppool.tile([P, H // 2, P], F16, tag="tp")
            for p in range(H // 2):
                nc.tensor.transpose(
                    tp[:, p, :],
                    rot.rearrange("p h d -> p (h d)")[:, p * P : (p + 1) * P],
                    identh,
                )
            nc.scalar.copy(dstT[:, :, ci * P : (ci + 1) * P], tp)

        def exchange(xch, sb_own, sb_peer, final_bars):
            """Scatter own rows into shared HBM, barrier, gather peer rows.
            All ordering is Tile dependency tracking: the indirect DMAs
            declare whole-tensor APs on xch, and the shuttle DMAs thread the
            barrier collectives into that chain."""
            nc.gpsimd.indirect_dma_start(
                out=xch, out_offset=bass.IndirectOffsetOnAxis(
                    ap=io_sb[:, 0:1], axis=0),
                in_=sb_own, in_offset=None,
            )
            if n_bars >= 1:
                nc.gpsimd.dma_start(out=bar_in,
                                    in_=xch[2 * P : 2 * P + 1, 0:16])
                nc.gpsimd.collective_compute(
                    "AllGather", mybir.AluOpType.bypass, replica_groups=ALL8,
                    ins=[bar_in], outs=[bar_out],
                )
                nc.gpsimd.dma_start(out=xch[2 * P : 2 * P + 1, 16:32],
                                    in_=bar_out[0:1, 0:16])
            nc.gpsimd.indirect_dma_start(
                out=sb_peer, out_offset=None,
                in_=xch, in_offset=bass.IndirectOffsetOnAxis(
                    ap=ip_sb[:, 0:1], axis=0),
            )
            if final_bars and n_bars >= 2:
                # reads done on both cores -> next repeat may overwrite slots
                for xch2, sbp in final_bars:
                    nc.gpsimd.dma_start(out=bar_in, in_=sbp)
                    nc.gpsimd.collective_compute(
                        "AllGather", mybir.AluOpType.bypass,
                        replica_groups=ALL8, ins=[bar_in], outs=[bar_out],
                    )
                    nc.gpsimd.dma_start(
                        out=xch2[2 * P : 2 * P + 1, 32:48],
                        in_=bar_out[0:1, 0:16])

        def scatter_only(xch, sb_own):
            nc.gpsimd.indirect_dma_start(
                out=xch, out_offset=bass.IndirectOffsetOnAxis(
                    ap=io_sb[:, 0:1], axis=0),
                in_=sb_own, in_offset=None,
            )

        def gather_only(xch, sb_peer):
            nc.gpsimd.indirect_dma_start(
                out=sb_peer, out_offset=None,
                in_=xch, in_offset=bass.IndirectOffsetOnAxis(
                    ap=ip_sb[:, 0:1], axis=0),
            )

        def merged_bar(dep_reads, xch_writes, col):
            """One 8-core barrier cc; ins gated on dep_reads (via bar_in
            shuttles), outs threaded into xch_writes scratch rows."""
            for dep in dep_reads:
                nc.gpsimd.dma_start(out=bar_in, in_=dep)
            nc.gpsimd.collective_compute(
                "AllGather", mybir.AluOpType.bypass, replica_groups=ALL8,
                ins=[bar_in], outs=[bar_out],
            )
            for xch in xch_writes:
                nc.gpsimd.dma_start(
                    out=xch[2 * P : 2 * P + 1, col : col + 16],
                    in_=bar_out[0:1, 0:16])

        for _rep in range(repeat):
            # ---- Phase A: K own half -> exchange; V own half -> exchange;
            #      Q own half. ----
            with ExitStack() as ph:
                wpool = ph.enter_context(tc.tile_pool(name="wA", bufs=1))
                pspool = ph.enter_context(
                    tc.tile_pool(name="psA", bufs=3, space="PSUM"))
                tppool = ph.enter_context(
                    tc.tile_pool(name="tpA", bufs=2, space="PSUM"))
                stg = ph.enter_context(tc.tile_pool(name="stgA", bufs=2))
                bq_sb = load_bias(pers, bq, "bq_sb") if use_bq else None
                bk_sb = load_bias(pers, bk, "bk_sb") if use_bk else None
                bv_sb = load_bias(pers, bv, "bv_sb") if use_bv else None

                for ci in range(NCQ):
                    psk = pspool.tile([P, D], F32, tag="ps")
                    proj_chunk(psk, ci * P, wk_first, bk_sb)
                    rotk = norm_rope(psk, cosq_sb[:, ci, :], sinq_sb[:, ci, :],
                                     knw_b, stg, nc.gpsimd, nc.gpsimd)
                    transpose_pairs(rotk, kT[:, 0], ci, tppool)

                vflat = v16.rearrange("p c h d -> p (c h d)")
                if n_bars == 3:
                    scatter_only(kxch, kTflat[:, 0:KROW])
                else:
                    exchange(kxch, kTflat[:, 0:KROW],
                             kTflat[:, KROW : 2 * KROW], None)

                wv_sb = wpool.tile([P, NDC, D], F16, tag="w")
                nc.sync.dma_start(out=wv_sb, in_=wv16)
                for ci in range(NCQ):
                    psv = pspool.tile([P, D], F32, tag="ps")
                    proj_chunk(psv, ci * P, wv_sb, bv_sb)
                    nc.scalar.copy(
                        v16[:, ci, :, 0:HD],
                        psv.rearrange("p (h d) -> p h d", h=H),
                    )

                if n_bars == 3:
                    scatter_only(vxch, vflat[:, 0:VROW])
                    # one barrier certifies both scatters on both cores
                    merged_bar(
                        [kxch[2 * P : 2 * P + 1, 0:16],
                         vxch[2 * P : 2 * P + 1, 0:16]],
                        [kxch, vxch], 16)
                    gather_only(vxch, vflat[:, VROW : 2 * VROW])
                    gather_only(kxch, kTflat[:, KROW : 2 * KROW])
                    # one trailing barrier protects slots across repeats
                    merged_bar(
                        [kTflat[0:1, 2 * KROW - 16 : 2 * KROW],
                         vflat[0:1, 2 * VROW - 16 : 2 * VROW]],
                        [kxch, vxch], 32)
                else:
                    exchange(
                        vxch, vflat[:, 0:VROW], vflat[:, VROW : 2 * VROW],
                        [(kxch, kTflat[0:1, 2 * KROW - 16 : 2 * KROW]),
                         (vxch, vflat[0:1, 2 * VROW - 16 : 2 * VROW])],
                    )

                wq_sb = wpool.tile([P, NDC, D], F16, tag="w")
                nc.sync.dma_start(out=wq_sb, in_=wq16)
                for ci in range(NCQ):
                    psq = pspool.tile([P, D], F32, tag="ps")
                    proj_chunk(psq, ci * P, wq_sb, bq_sb)
                    rotq = norm_rope(psq, cosq_sb[:, ci, :], sinq_sb[:, ci, :],
                                     qnw_b, stg, nc.vector, nc.vector)
                    transpose_pairs(rotq, qT, ci, tppool)

            # ---- Phase B: attention ----
            with ExitStack() as ph:
                spool = ph.enter_context(
                    tc.tile_pool(name="sB", bufs=2, space="PSUM"))
                upool = ph.enter_context(
                    tc.tile_pool(name="uB", bufs=2, space="PSUM"))
                ppool = ph.enter_context(tc.tile_pool(name="ptB", bufs=4))
                bcpool = ph.enter_context(tc.tile_pool(name="bcB", bufs=2))

                def s_chunk(h, c, sT):
                    pi, po = h // 2, (h % 2) * HD
                    for n0 in range(0, LQ, 512):
                        nc.tensor.matmul(
                            sT[:, n0 : n0 + 512],
                            kT[po : po + HD, c // NCQ, pi,
                               (c % NCQ) * P : (c % NCQ + 1) * P],
                            qT[po : po + HD, pi, n0 : n0 + 512],
                            start=True,
                            stop=True,
                        )

                def av_chunk(h, c, pt, U):
                    for n0 in range(0, LQ, 512):
                        nc.tensor.matmul(
                            U[:, n0 : n0 + 512],
                            v16[:, c, h, :],
                            pt[:, n0 : n0 + 512],
                            start=(c == 0),
                            stop=(c == NCK - 1),
                        )

                for h in range(H):
                    pi = h // 2
                    U = upool.tile([HD + 1, LQ], F32, tag="U")
                    pt_prev = None
                    for c in range(NCK):
                        sT = spool.tile([P, LQ], F32, tag="sT")
                        s_chunk(h, c, sT)
                        pt = ppool.tile([P, LQ], F16, tag="pt")
                        nc.scalar.activation(pt, sT, AF.Exp, scale=0.125)
                        if pt_prev is not None:
                            av_chunk(h, c - 1, pt_prev, U)
                        pt_prev = pt
                    av_chunk(h, NCK - 1, pt_prev, U)
                    po = (h % 2) * HD
                    nc.scalar.copy(attnT[po : po + HD, pi, :], U[0:HD, :])
                    inv = bcpool.tile([1, LQ], F32, tag="inv")
                    nc.vector.reciprocal(inv, U[HD : HD + 1, :])
                    nc.sync.dma_start(out=invstage[h, :], in_=inv)
                    if h % 2 == 1:
                        bc = bcpool.tile([P, LQ], F32, tag="bc")
                        for hh in range(2):
                            iv = invstage[2 * pi + hh, :]
                            nc.sync.dma_start(
                                out=bc[hh * HD : (hh + 1) * HD, :],
                                in_=bass.AP(tensor=iv.tensor, offset=iv.offset,
                                            ap=[[0, HD], [1, LQ]]),
                            )
                        nc.vector.tensor_mul(attnT[:, pi, :], attnT[:, pi, :], bc)

            # ---- Phase C: out projection ----
            with ExitStack() as ph:
                opool = ph.enter_context(
                    tc.tile_pool(name="oC", bufs=2, space="PSUM"))
                obpool = ph.enter_context(tc.tile_pool(name="obC", bufs=2))
                # reuse the K-weights buffer for Wout (disjoint lifetimes)
                wout_sb = wk_first
                nc.sync.dma_start(out=wout_sb, in_=wob)
                for cj in range(NCQ):
                    pso = opool.tile([P, D], F32, tag="pso")
                    for n0 in range(0, D, 512):
                        for j in range(NDC):
                            nc.tensor.matmul(
                                pso[:, n0 : n0 + 512],
                                attnT[:, j, cj * P : (cj + 1) * P],
                                wout_sb[:, j, n0 : n0 + 512],
                                start=(j == 0),
                                stop=(j == NDC - 1),
                            )
                    ob = obpool.tile([P, D], F32, tag="ob")
                    if use_bout:
                        nc.vector.tensor_add(ob, pso, bout_b)
                    else:
                        nc.scalar.copy(ob[:, 0 : D // 2], pso[:, 0 : D // 2])
                        nc.vector.tensor_copy(ob[:, D // 2 : D], pso[:, D // 2 : D])
                    nc.sync.dma_start(out=out[cj * P : (cj + 1) * P, :], in_=ob)

    return nc


_PROGRAM_CACHE = {}


def _get_program(flags, repeat=1, n_bars=2):
    key = (flags, repeat, n_bars)
    if key not in _PROGRAM_CACHE:
        _patch_tile_drain()
        _PROGRAM_CACHE[key] = _build_program(*flags, repeat=repeat,
                                             n_bars=n_bars)
    return _PROGRAM_CACHE[key]


def _rope_tables():
    pos = np.arange(L, dtype=np.float32)
    inv_freq = (1.0 / (THETA ** (np.arange(0, HD, 2, dtype=np.float32) / HD))
                ).astype(np.float32)
    ang = pos[:, None] * inv_freq[None, :]
    return np.cos(ang).astype(np.float32), np.sin(ang).astype(np.float32)


def _chunked_pf(t, nch):
    """[nch*128, F] -> [128, nch, F] (partition-major chunk layout)."""
    return np.ascontiguousarray(
        t.reshape(nch, P, -1).transpose(1, 0, 2))


def _w16_layout(w):
    """[D, N] f32 -> [128, NDC, N] fp16."""
    return np.ascontiguousarray(
        w.reshape(NDC, P, -1).transpose(1, 0, 2)).astype(np.float16)


def _make_in_maps(x, Wqkv, bqkv, qn_w, kn_w, Wout, bout, flags):
    use_bq, use_bk, use_bv, use_bout, use_qnw, use_knw = flags
    cos, sin = _rope_tables()
    cosh = cos.astype(np.float16)
    sinh = sin.astype(np.float16)
    wq16 = _w16_layout(Wqkv[:, 0:D])
    wk16 = _w16_layout(Wqkv[:, D : 2 * D])
    wv16 = _w16_layout(Wqkv[:, 2 * D : 3 * D])
    wob = _w16_layout(Wout)
    base_idx = np.arange(P, dtype=np.int32).reshape(P, 1)
    in_maps = []
    for c in range(N_CORES):
        b, half = c // 2, c % 2
        # roll the sequence so this core's own rows are rows [0, LQ);
        # rope tables roll identically (softmax over k is order-invariant)
        xr = np.roll(x[b], -half * LQ, axis=0)[0:LQ]
        cosr = np.roll(cosh, -half * LQ, axis=0)[0:LQ]
        sinr = np.roll(sinh, -half * LQ, axis=0)[0:LQ]
        xt16 = np.ascontiguousarray(
            xr.T.reshape(NDC, P, LQ).transpose(1, 0, 2)).astype(np.float16)
        m = {
            "xt16": xt16,
            "wq16": wq16,
            "wk16": wk16,
            "wv16": wv16,
            "wob": wob,
            "cosq": _chunked_pf(cosr, NCQ),
            "sinq": _chunked_pf(sinr, NCQ),
            "idx_own": base_idx + half * P,
            "idx_peer": base_idx + (1 - half) * P,
        }
        if use_bq:
            m["bq"] = np.ascontiguousarray(bqkv[0:D]).reshape(1, D)
        if use_bk:
            m["bk"] = np.ascontiguousarray(bqkv[D : 2 * D]).reshape(1, D)
        if use_bv:
            m["bv"] = np.ascontiguousarray(bqkv[2 * D : 3 * D]).reshape(1, D)
        if use_bout:
            m["bout"] = np.ascontiguousarray(bout).reshape(1, D)
        if use_qnw:
            m["qnw"] = np.ascontiguousarray(qn_w).reshape(1, HD)
        if use_knw:
            m["knw"] = np.ascontiguousarray(kn_w).reshape(1, HD)
        in_maps.append(m)
    return in_maps


def _flags_for(bqkv, qn_w, kn_w, bout):
    return (
        bool(np.any(bqkv[0:D])),
        bool(np.any(bqkv[D : 2 * D])),
        bool(np.any(bqkv[2 * D : 3 * D])),
        bool(np.any(bout)),
        bool(np.any(qn_w != 1.0)),
        bool(np.any(kn_w != 1.0)),
    )


def _assemble(results):
    out = np.empty((B, L, D), dtype=np.float32)
    for c in range(N_CORES):
        b, half = c // 2, c % 2
        out[b, half * LQ : (half + 1) * LQ, :] = results[c]["out"]
    return out


def kernel(x, Wqkv, bqkv, qn_w, kn_w, Wout, bout, _trace=False):
    x = np.asarray(x, dtype=np.float32)
    Wqkv = np.asarray(Wqkv, dtype=np.float32)
    bqkv = np.asarray(bqkv, dtype=np.float32)
    qn_w = np.asarray(qn_w, dtype=np.float32)
    kn_w = np.asarray(kn_w, dtype=np.float32)
    Wout = np.asarray(Wout, dtype=np.float32)
    bout = np.asarray(bout, dtype=np.float32)

    flags = _flags_for(bqkv, qn_w, kn_w, bout)
    nc = _get_program(flags)
    if not getattr(nc, "_waits_split", False):
        _split_waits(nc)
        nc._waits_split = True
    in_maps = _make_in_maps(x, Wqkv, bqkv, qn_w, kn_w, Wout, bout, flags)
    res = bass_utils.run_bass_kernel_spmd(
        nc, in_maps, core_ids=list(range(N_CORES))
    )
    out = _assemble(res.results)
    if _trace:
        return out, res
    return out
